# revision 22
# baseline (speedup 1.0000x reference)
"""DiffuseRouter kernel for 8 TRN2 NeuronCores.

Reference computation (enable_time=False, soft_time_routing=True):
    out[b, l, d] = (1/3) * sum_g sum_e expert_emb_g[e, b, l, d]
i.e. a uniform-weighted sum of 28 expert planes per batch element.

Sharding: pure data-parallel over batch B=8 -> one batch element per core.
Each core reads its 28 [256, 1280] f32 planes (36.7 MB), reduces them
on-chip, scales by 1/3, and writes its [256, 1280] output.  No collectives.

v16 = window-major streaming, all-uniform loads, deep descriptor queue.
Hard-won trace laws (see the session probes):
  * Only [128, N] dma_starts with a contiguous DRAM block run at line rate
    (~25.9 GB/s per SDMA engine, 413 GB/s wall).  Partial-partition loads
    ([32..124, N]) run at HALF rate (12.5-15 GB/s per engine) -- probe
    measured; that sank the engine-15-deload schemes (v9-v11).  Strided
    DRAM sources are worse still (12 GB/s).  nc.gpsimd.dma_start (SWDGE)
    hangs the device in this runtime.
  * HWDGE deals one dma_start's descriptors to the 16 SDMA engines in
    contiguous chunks of ceil(n/16) from engine 0: a [128, N] load maps
    partitions 8k..8k+7 -> engine k.
  * An all-PE reduction (tensor engine ~44% active, 145 matmuls) trips
    the activity throttle (util cap 0.5) and drags the DMA fabric ~13%.
    The split used here (PE 3 PSUM banks + DVE for 1024 cols, ~33%+28%
    active) stays clean.
  * SDMA engine 15 has an intermittent whole-run ~21.5 GB/s mode (other
    engines unaffected); it cannot be deloaded with uniform loads.  With
    bufs=8 keeping ~64 descriptors queued per engine the observed rate
    was 5/6 fast runs (~106 us) vs 1/6 slow (~123 us).
Layout: column-window-major streams so output stores overlap the load
stream instead of trailing it.  PE windows w=0,1,2 (cols w*512..w*512+512
of every plane) are host-packed as contiguous chunk strings and
accumulated into PSUM bank w by 28 identity matmuls (fp32r); each bank
stops, scales x1/3 through ACT, and stores while later streams are still
loading.  The DVE zone (cols 1536:2560 of every plane) streams as its own
chunk string, accumulated by DVE scalar_tensor_tensor into SBUF, scaled
and stored at the end.  Load tiles interleave PE/DVE streams 2:1 so both
consumers stay fed; the final stream tapers to 2/1/1-chunk tiles so the
post-landing serial matmul tail is <1 us.
"""

import numpy as np

import concourse.bacc as bacc
import concourse.tile as tile
from concourse import mybir
from concourse.alu_op_type import AluOpType
from concourse.bass_utils import run_bass_kernel_spmd

N_CORES = 8
E_TOTAL = 28  # 4 + 8 + 16 experts across the 3 granularity levels
L, D = 256, 1280
P = 128  # SBUF partitions
FD = (L // P) * D  # 2560 free-dim elements per partition per plane
BW = 512  # one 2 KB PSUM bank of f32
NB_PE = 3  # PE windows (cols 0..1536)
DVE_LO = NB_PE * BW  # 1536
DVE_W = FD - DVE_LO  # 1024 cols accumulated on DVE
SCALE = 1.0 / 3.0

# Tile schedule: (kind, stream, chunk_lo, chunk_hi).  kind 'P' tiles hold
# chunks of 512 cols for PE stream `stream`; kind 'V' tiles hold chunks of
# 1024 cols for the DVE zone.  PE tiles carry 10/10/8 chunks, DVE tiles
# 5/3 chunks (20,480/16,384 B descriptors); with bufs=8 this keeps ~64
# descriptors queued per engine.  Interleaved ~2:1 so both consumers stay
# fed; all loads are full-128-partition contiguous blocks.
SCHED = [
    ("P", 0, 0, 10), ("V", 0, 0, 5), ("P", 0, 10, 20), ("V", 0, 5, 10),
    ("P", 0, 20, 28), ("V", 0, 10, 15), ("P", 1, 0, 10), ("V", 0, 15, 20),
    ("P", 1, 10, 20), ("V", 0, 20, 25), ("P", 1, 20, 28), ("V", 0, 25, 28),
    # Last stream tapers to tiny tiles: consumption is gated per-tile on
    # the tile's LAST descriptor, so the final tiles carry 2/1/1 chunks to
    # shrink the post-landing serial matmul tail to <1 us.
    ("P", 2, 0, 10), ("P", 2, 10, 20), ("P", 2, 20, 24),
    ("P", 2, 24, 26), ("P", 2, 26, 27), ("P", 2, 27, 28),
]

_NC_CACHE = None


def _build_nc():
    """Build the SPMD Bass program (identical on all 8 cores)."""
    nc = bacc.Bacc(
        "TRN2", target_bir_lowering=False, debug=False, enable_partition_id=False
    )
    f32 = mybir.dt.float32
    f32r = mybir.dt.float32r

    xs, xs_r = [], []
    for i, (kind, s, lo, hi) in enumerate(SCHED):
        cw = BW if kind == "P" else DVE_W
        t = nc.dram_tensor(f"x{i}", [P, (hi - lo) * cw], f32,
                           kind="ExternalInput")
        xs.append(t)
        xs_r.append(t.ap().bitcast(f32r))
    ident_d = nc.dram_tensor("ident", [P, P], f32, kind="ExternalInput")
    out_pe = [
        nc.dram_tensor(f"out{w}", [P, BW], f32, kind="ExternalOutput")
        for w in range(NB_PE)
    ]
    out_dve = nc.dram_tensor("outv", [P, DVE_W], f32, kind="ExternalOutput")

    with tile.TileContext(nc) as tc:
        with (
            tc.tile_pool(name="in", bufs=8) as pin,
            tc.tile_pool(name="const", bufs=1) as pconst,
            tc.tile_pool(name="acc", bufs=1) as pacc,
            tc.tile_pool(name="ps", bufs=1, space="PSUM") as pps,
        ):
            ident = pconst.tile([P, P], f32r, name="ident", tag="ident")
            # Identity rides the ACT ring; the sync ring carries only loads.
            nc.scalar.dma_start(out=ident[:], in_=ident_d.ap().bitcast(f32r))

            psums = [
                pps.tile([P, BW], f32, name=f"ps{w}", tag=f"ps{w}")
                for w in range(NB_PE)
            ]
            souts = pacc.tile([P, NB_PE * BW], f32, name="souts", tag="souts")
            acc = pacc.tile([P, DVE_W], f32, name="acc", tag="acc")
            vout = pacc.tile([P, DVE_W], f32, name="vout", tag="vout")

            mult = AluOpType.mult
            add = AluOpType.add
            n_v = 0

            for i, (kind, s, lo, hi) in enumerate(SCHED):
                cw = BW if kind == "P" else DVE_W
                t = pin.tile([P, (hi - lo) * cw], f32r)
                nc.sync.dma_start(out=t[:], in_=xs_r[i])
                if kind == "P":
                    for c in range(lo, hi):
                        nc.tensor.matmul(
                            psums[s][:], ident[:],
                            t[:, (c - lo) * BW : (c - lo + 1) * BW],
                            start=(c == 0), stop=(c == E_TOTAL - 1),
                        )
                    if hi == E_TOTAL:
                        # Bank s complete: scale + store while later
                        # streams are still loading.
                        ws = slice(s * BW, (s + 1) * BW)
                        nc.scalar.mul(souts[:, ws], psums[s][:], SCALE)
                        nc.scalar.dma_start(
                            out=out_pe[s].ap(), in_=souts[:, ws]
                        )
                else:
                    for c in range(lo, hi):
                        src = t[
                            :, (c - lo) * DVE_W : (c - lo + 1) * DVE_W
                        ].bitcast(f32)
                        if c == 0:
                            nc.vector.tensor_scalar_mul(acc[:], src, 1.0)
                        else:
                            nc.vector.scalar_tensor_tensor(
                                acc[:], src, 1.0, acc[:], mult, add
                            )
                    n_v += hi - lo
                    if hi == E_TOTAL:
                        # DVE zone complete: scale on ACT and store.
                        nc.scalar.mul(vout[:], acc[:], SCALE)
                        nc.scalar.dma_start(out=out_dve.ap(), in_=vout[:])
    nc.compile()
    return nc


def _get_nc():
    global _NC_CACHE
    if _NC_CACHE is None:
        _NC_CACHE = _build_nc()
    return _NC_CACHE


def _pack_core(v):
    """v: [28, 128, 2560] planes for one batch element -> input map."""
    im = {"ident": np.eye(P, dtype=np.float32)}
    for i, (kind, s, lo, hi) in enumerate(SCHED):
        if kind == "P":
            blk = v[lo:hi, :, s * BW : (s + 1) * BW]  # [n, 128, 512]
        else:
            blk = v[lo:hi, :, DVE_LO:FD]  # [n, 128, 1024]
        im[f"x{i}"] = np.ascontiguousarray(
            blk.transpose(1, 0, 2).reshape(P, -1)
        )
    return im


def _run(inputs, trace=False, trace_kwargs=None):
    e0 = np.asarray(inputs["expert_emb_0"], dtype=np.float32)
    e1 = np.asarray(inputs["expert_emb_1"], dtype=np.float32)
    e2 = np.asarray(inputs["expert_emb_2"], dtype=np.float32)
    B = e0.shape[1]
    assert B == N_CORES, f"expected B == {N_CORES}, got {B}"

    in_maps = []
    for b in range(B):
        xb_full = np.concatenate([e0[:, b], e1[:, b], e2[:, b]], axis=0)
        v = xb_full.reshape(E_TOTAL, P, FD)
        in_maps.append(_pack_core(v))

    kw = {}
    if trace:
        kw["trace"] = True
        if trace_kwargs:
            kw.update(trace_kwargs)
    try:
        res = run_bass_kernel_spmd(_get_nc(), in_maps, list(range(N_CORES)), **kw)
    except Exception:
        # One retry: transient device errors usually clear on re-dispatch.
        res = run_bass_kernel_spmd(_get_nc(), in_maps, list(range(N_CORES)), **kw)
    outs = []
    for b in range(B):
        full = np.concatenate(
            [res.results[b][f"out{w}"] for w in range(NB_PE)]
            + [res.results[b]["outv"]],
            axis=1,
        )
        outs.append(full.reshape(L, D))
    out = np.stack(outs, axis=0)
    return out.astype(np.float32, copy=False), res


def kernel(**inputs) -> np.ndarray:
    out, _ = _run(inputs, trace=False)
    return out


# revision 23
# speedup vs baseline: 1.3471x; 1.3471x over previous
"""DiffuseRouter kernel for 8 TRN2 NeuronCores.

Reference computation (enable_time=False, soft_time_routing=True):
    out[b, l, d] = (1/3) * sum_g sum_e expert_emb_g[e, b, l, d]
i.e. a uniform-weighted sum of 28 expert planes per batch element.

Sharding: pure data-parallel over batch B=8 -> one batch element per core.
No collectives.

v17 = fp16 load path.  The harness gate is rel_err < 2e-2; converting the
expert planes to fp16 during host packing HALVES the HBM traffic of this
memory-bound reduction (36.7 -> 18.3 MB per core) at a measured ~1e-3
relative error (inputs are N(0,1); fp16 quantization is ~5e-4 relative
per element, root-sum-squared over 28 addends).  PSUM accumulation stays
f32; the output is f32.

Trace laws carried over from the f32 builds (probe-verified):
  * Only [128, N] dma_starts with a contiguous DRAM block run at line
    rate (~25.9 GB/s per SDMA engine); partial-partition loads run at
    half rate; strided sources at 12 GB/s; SWDGE hangs the device.
  * HWDGE deals descriptors in ceil(n/16) chunks from engine 0, so
    [128, N] maps partitions 8k..8k+7 -> engine k.  SDMA engine 15 has
    an intermittent whole-run ~21.5 GB/s mode; it cannot be deloaded.
  * Tensor-engine activity ~44% trips the throttle (util cap 0.5) and
    drags the DMA fabric; keep PE + DVE both under ~40% active.
  * Consumption is gated per-tile on the tile's last descriptor: taper
    the final tiles to 1-2 chunks.
Structure: column-window-major streams so output stores overlap the load
stream.  PE windows w=0,1 (cols 0:1024) are host-packed as contiguous
fp16 chunk strings and accumulated into PSUM banks by 28 identity
matmuls each (f16 x f16 -> f32 PSUM); each bank stops, scales x1/3
through ACT, and stores.  The DVE zone (cols 1024:2560) streams as its
own chunk string, accumulated by DVE scalar_tensor_tensor in f16 (2x
throughput for 16-bit), scaled to f32 on ACT and stored mid-stream.
"""

import numpy as np

import concourse.bacc as bacc
import concourse.tile as tile
from concourse import mybir
from concourse.alu_op_type import AluOpType
from concourse.bass_utils import run_bass_kernel_spmd

N_CORES = 8
E_TOTAL = 28  # 4 + 8 + 16 experts across the 3 granularity levels
L, D = 256, 1280
P = 128  # SBUF partitions
FD = (L // P) * D  # 2560 free-dim elements per partition per plane
BW = 512  # one 2 KB PSUM bank of f32
NB_PE = 2  # PE windows (cols 0..1024)
DVE_LO = NB_PE * BW  # 1024
DVE_W = FD - DVE_LO  # 1536 cols accumulated on DVE (f16)
SCALE = 1.0 / 3.0

# Tile schedule: (kind, stream, chunk_lo, chunk_hi).  'P' tiles hold fp16
# chunks of 512 cols (1024 B) for PE stream `stream`; 'V' tiles hold fp16
# chunks of 1536 cols (3072 B) for the DVE zone.  Descriptor sizes stay
# in the proven 8-20 KB band; the final stream tapers to tiny tiles.
SCHED = [
    ("P", 0, 0, 20), ("V", 0, 0, 6), ("V", 0, 6, 12), ("P", 0, 20, 28),
    ("V", 0, 12, 18), ("P", 1, 0, 20), ("V", 0, 18, 24), ("V", 0, 24, 28),
    ("P", 1, 20, 24), ("P", 1, 24, 26), ("P", 1, 26, 27), ("P", 1, 27, 28),
]

_NC_CACHE = None


def _build_nc():
    """Build the SPMD Bass program (identical on all 8 cores)."""
    nc = bacc.Bacc(
        "TRN2", target_bir_lowering=False, debug=False, enable_partition_id=False
    )
    f32 = mybir.dt.float32
    f16 = mybir.dt.float16

    xs = []
    for i, (kind, s, lo, hi) in enumerate(SCHED):
        cw = BW if kind == "P" else DVE_W
        xs.append(
            nc.dram_tensor(f"x{i}", [P, (hi - lo) * cw], f16,
                           kind="ExternalInput")
        )
    ident_d = nc.dram_tensor("ident", [P, P], f16, kind="ExternalInput")
    out_pe = [
        nc.dram_tensor(f"out{w}", [P, BW], f32, kind="ExternalOutput")
        for w in range(NB_PE)
    ]
    out_dve = nc.dram_tensor("outv", [P, DVE_W], f32, kind="ExternalOutput")

    with tile.TileContext(nc) as tc:
        with (
            tc.tile_pool(name="in", bufs=8) as pin,
            tc.tile_pool(name="const", bufs=1) as pconst,
            tc.tile_pool(name="acc", bufs=1) as pacc,
            tc.tile_pool(name="ps", bufs=1, space="PSUM") as pps,
        ):
            ident = pconst.tile([P, P], f16, name="ident", tag="ident")
            # Identity rides the ACT ring; the sync ring carries only loads.
            nc.scalar.dma_start(out=ident[:], in_=ident_d.ap())

            psums = [
                pps.tile([P, BW], f32, name=f"ps{w}", tag=f"ps{w}")
                for w in range(NB_PE)
            ]
            souts = pacc.tile([P, NB_PE * BW], f32, name="souts", tag="souts")
            acc = pacc.tile([P, DVE_W], f16, name="acc", tag="acc")
            vout = pacc.tile([P, DVE_W], f32, name="vout", tag="vout")

            mult = AluOpType.mult
            add = AluOpType.add

            with nc.allow_low_precision(
                reason="fp16 DVE accumulation; harness gate is 2e-2"
            ):
                for i, (kind, s, lo, hi) in enumerate(SCHED):
                    cw = BW if kind == "P" else DVE_W
                    t = pin.tile([P, (hi - lo) * cw], f16)
                    nc.sync.dma_start(out=t[:], in_=xs[i].ap())
                    if kind == "P":
                        for c in range(lo, hi):
                            nc.tensor.matmul(
                                psums[s][:], ident[:],
                                t[:, (c - lo) * BW : (c - lo + 1) * BW],
                                start=(c == 0), stop=(c == E_TOTAL - 1),
                            )
                        if hi == E_TOTAL:
                            ws = slice(s * BW, (s + 1) * BW)
                            nc.scalar.mul(souts[:, ws], psums[s][:], SCALE)
                            nc.scalar.dma_start(
                                out=out_pe[s].ap(), in_=souts[:, ws]
                            )
                    else:
                        for c in range(lo, hi):
                            src = t[:, (c - lo) * DVE_W : (c - lo + 1) * DVE_W]
                            if c == 0:
                                nc.vector.tensor_scalar_mul(acc[:], src, 1.0)
                            else:
                                nc.vector.scalar_tensor_tensor(
                                    acc[:], src, 1.0, acc[:], mult, add
                                )
                        if hi == E_TOTAL:
                            # DVE zone complete: f16 acc -> f32, x1/3, store.
                            nc.scalar.mul(vout[:], acc[:], SCALE)
                            nc.scalar.dma_start(out=out_dve.ap(), in_=vout[:])
    nc.compile()
    return nc


def _get_nc():
    global _NC_CACHE
    if _NC_CACHE is None:
        _NC_CACHE = _build_nc()
    return _NC_CACHE


def _pack_core(v16):
    """v16: [28, 128, 2560] fp16 planes for one batch element -> input map."""
    im = {"ident": np.eye(P, dtype=np.float16)}
    for i, (kind, s, lo, hi) in enumerate(SCHED):
        if kind == "P":
            blk = v16[lo:hi, :, s * BW : (s + 1) * BW]  # [n, 128, 512]
        else:
            blk = v16[lo:hi, :, DVE_LO:FD]  # [n, 128, 1536]
        im[f"x{i}"] = np.ascontiguousarray(
            blk.transpose(1, 0, 2).reshape(P, -1)
        )
    return im


def _run(inputs, trace=False, trace_kwargs=None):
    e0 = np.asarray(inputs["expert_emb_0"], dtype=np.float32)
    e1 = np.asarray(inputs["expert_emb_1"], dtype=np.float32)
    e2 = np.asarray(inputs["expert_emb_2"], dtype=np.float32)
    B = e0.shape[1]
    assert B == N_CORES, f"expected B == {N_CORES}, got {B}"

    in_maps = []
    for b in range(B):
        xb_full = np.concatenate([e0[:, b], e1[:, b], e2[:, b]], axis=0)
        v16 = xb_full.reshape(E_TOTAL, P, FD).astype(np.float16)
        in_maps.append(_pack_core(v16))

    kw = {}
    if trace:
        kw["trace"] = True
        if trace_kwargs:
            kw.update(trace_kwargs)
    try:
        res = run_bass_kernel_spmd(_get_nc(), in_maps, list(range(N_CORES)), **kw)
    except Exception:
        # One retry: transient device errors usually clear on re-dispatch.
        res = run_bass_kernel_spmd(_get_nc(), in_maps, list(range(N_CORES)), **kw)
    outs = []
    for b in range(B):
        full = np.concatenate(
            [res.results[b][f"out{w}"] for w in range(NB_PE)]
            + [res.results[b]["outv"]],
            axis=1,
        )
        outs.append(full.reshape(L, D))
    out = np.stack(outs, axis=0)
    return out.astype(np.float32, copy=False), res


def kernel(**inputs) -> np.ndarray:
    out, _ = _run(inputs, trace=False)
    return out


# revision 24
# speedup vs baseline: 1.7328x; 1.2863x over previous
"""DiffuseRouter kernel for 8 TRN2 NeuronCores.

Reference computation (enable_time=False, soft_time_routing=True):
    out[b, l, d] = (1/3) * sum_g sum_e expert_emb_g[e, b, l, d]
i.e. a uniform-weighted sum of 28 expert planes per batch element.

Sharding: pure data-parallel over batch B=8 -> one batch element per core.
No collectives.

v18 = fp16 load path + PE/DVE column split balanced to their measured
rates.  The harness gate is rel_err < 2e-2; converting the expert planes
to fp16 during host packing HALVES the HBM traffic of this memory-bound
reduction (36.7 -> 18.3 MB per core) at ~7e-4 measured relative error.
PSUM accumulation stays f32; the output is f32.

Measured rates that set the split (v17 trace):
  * DMA: ~26 GB/s per SDMA engine -> 44 us (53 us when engine 15 is in
    its intermittent slow mode) for the 1.15 MB/engine fp16 stream.
  * PE fp16 matmul [128, 512]: ~756 ns (0.68 cols/ns incl. LDWEIGHTS).
  * DVE scalar_tensor_tensor accumulate: 2R1W port-bound at ~0.73
    cols/ns regardless of dtype (single-source ops hit 2.3x that).
  Neither engine alone covers 28 planes x 2560 cols (~105 us); split
  PE=1280 cols (banks 512+512+256) and DVE=1280 cols -> ~54 us each,
  overlapping the 44-53 us load stream.
Carried-over trace laws: only [128, N] contiguous-source dma_starts run
at line rate (partial-partition = half rate, strided = 12 GB/s, SWDGE
hangs); descriptors deal in ceil(n/16) chunks from engine 0; tensor
activity ~44% trips the throttle; per-tile consumption gating means the
final stream tapers to 1-chunk tiles.  Window-major streams let each
PSUM bank stop, scale x1/3 on ACT, and store while later columns load.
"""

import numpy as np

import concourse.bacc as bacc
import concourse.tile as tile
from concourse import mybir
from concourse.alu_op_type import AluOpType
from concourse.bass_utils import run_bass_kernel_spmd

N_CORES = 8
E_TOTAL = 28  # 4 + 8 + 16 experts across the 3 granularity levels
L, D = 256, 1280
P = 128  # SBUF partitions
FD = (L // P) * D  # 2560 free-dim elements per partition per plane
SCALE = 1.0 / 3.0

# Column streams: PE PSUM banks 0/1/2 take cols 0:512/512:1024/1024:1280;
# DVE accumulates cols 1280:2560 in f16.
STREAMS = {0: (0, 512), 1: (512, 1024), 2: (1024, 1280), "V": (1280, 2560)}

# Tile schedule: (stream, chunk_lo, chunk_hi); chunk c of stream s is
# plane c's column block.  Small lead-in tiles start both engines early;
# the final PE stream tapers to 1-chunk tiles (consumption is gated on a
# tile's last descriptor).  All loads are [128, N] contiguous blocks.
SCHED = [
    (0, 0, 4), ("V", 0, 2), (0, 4, 16), ("V", 2, 7), (1, 0, 12),
    ("V", 7, 12), (0, 16, 28), (1, 12, 24), ("V", 12, 18), (2, 0, 14),
    ("V", 18, 24), (1, 24, 28), ("V", 24, 28), (2, 14, 22), (2, 22, 26),
    (2, 26, 27), (2, 27, 28),
]

_NC_CACHE = None


def _build_nc():
    """Build the SPMD Bass program (identical on all 8 cores)."""
    nc = bacc.Bacc(
        "TRN2", target_bir_lowering=False, debug=False, enable_partition_id=False
    )
    f32 = mybir.dt.float32
    f16 = mybir.dt.float16

    xs = []
    for i, (s, lo, hi) in enumerate(SCHED):
        c0, c1 = STREAMS[s]
        xs.append(
            nc.dram_tensor(f"x{i}", [P, (hi - lo) * (c1 - c0)], f16,
                           kind="ExternalInput")
        )
    ident_d = nc.dram_tensor("ident", [P, P], f16, kind="ExternalInput")
    outs_d = {
        s: nc.dram_tensor(f"out{s}", [P, STREAMS[s][1] - STREAMS[s][0]],
                          f32, kind="ExternalOutput")
        for s in STREAMS
        if s != "V"
    }
    outv_d = nc.dram_tensor("outv", [P, STREAMS["V"][1] - STREAMS["V"][0]],
                            f32, kind="ExternalOutput")

    with tile.TileContext(nc) as tc:
        with (
            tc.tile_pool(name="in", bufs=8) as pin,
            tc.tile_pool(name="const", bufs=1) as pconst,
            tc.tile_pool(name="acc", bufs=1) as pacc,
            tc.tile_pool(name="ps", bufs=1, space="PSUM") as pps,
        ):
            ident = pconst.tile([P, P], f16, name="ident", tag="ident")
            # Identity rides the ACT ring; the sync ring carries only loads.
            nc.scalar.dma_start(out=ident[:], in_=ident_d.ap())

            psums = {
                s: pps.tile([P, STREAMS[s][1] - STREAMS[s][0]], f32,
                            name=f"ps{s}", tag=f"ps{s}")
                for s in (0, 1, 2)
            }
            souts = {
                s: pacc.tile([P, STREAMS[s][1] - STREAMS[s][0]], f32,
                             name=f"so{s}", tag=f"so{s}")
                for s in (0, 1, 2)
            }
            vw = STREAMS["V"][1] - STREAMS["V"][0]
            acc = pacc.tile([P, vw], f16, name="acc", tag="acc")
            vout = pacc.tile([P, vw], f32, name="vout", tag="vout")

            mult = AluOpType.mult
            add = AluOpType.add

            with nc.allow_low_precision(
                reason="fp16 DVE accumulation; harness gate is 2e-2"
            ):
                for i, (s, lo, hi) in enumerate(SCHED):
                    w = STREAMS[s][1] - STREAMS[s][0]
                    t = pin.tile([P, (hi - lo) * w], f16)
                    nc.sync.dma_start(out=t[:], in_=xs[i].ap())
                    if s == "V":
                        for c in range(lo, hi):
                            src = t[:, (c - lo) * w : (c - lo + 1) * w]
                            if c == 0:
                                nc.vector.tensor_scalar_mul(acc[:], src, 1.0)
                            else:
                                nc.vector.scalar_tensor_tensor(
                                    acc[:], src, 1.0, acc[:], mult, add
                                )
                        if hi == E_TOTAL:
                            # DVE zone complete: f16 acc -> f32, x1/3, store.
                            nc.scalar.mul(vout[:], acc[:], SCALE)
                            nc.scalar.dma_start(out=outv_d.ap(), in_=vout[:])
                    else:
                        for c in range(lo, hi):
                            nc.tensor.matmul(
                                psums[s][:], ident[:],
                                t[:, (c - lo) * w : (c - lo + 1) * w],
                                start=(c == 0), stop=(c == E_TOTAL - 1),
                            )
                        if hi == E_TOTAL:
                            nc.scalar.mul(souts[s][:], psums[s][:], SCALE)
                            nc.scalar.dma_start(
                                out=outs_d[s].ap(), in_=souts[s][:]
                            )
    nc.compile()
    return nc


def _get_nc():
    global _NC_CACHE
    if _NC_CACHE is None:
        _NC_CACHE = _build_nc()
    return _NC_CACHE


def _pack_core(v16):
    """v16: [28, 128, 2560] fp16 planes for one batch element -> input map."""
    im = {"ident": np.eye(P, dtype=np.float16)}
    for i, (s, lo, hi) in enumerate(SCHED):
        c0, c1 = STREAMS[s]
        blk = v16[lo:hi, :, c0:c1]  # [n, 128, w]
        im[f"x{i}"] = np.ascontiguousarray(
            blk.transpose(1, 0, 2).reshape(P, -1)
        )
    return im


def _run(inputs, trace=False, trace_kwargs=None):
    e0 = np.asarray(inputs["expert_emb_0"], dtype=np.float32)
    e1 = np.asarray(inputs["expert_emb_1"], dtype=np.float32)
    e2 = np.asarray(inputs["expert_emb_2"], dtype=np.float32)
    B = e0.shape[1]
    assert B == N_CORES, f"expected B == {N_CORES}, got {B}"

    in_maps = []
    for b in range(B):
        xb_full = np.concatenate([e0[:, b], e1[:, b], e2[:, b]], axis=0)
        v16 = xb_full.reshape(E_TOTAL, P, FD).astype(np.float16)
        in_maps.append(_pack_core(v16))

    kw = {}
    if trace:
        kw["trace"] = True
        if trace_kwargs:
            kw.update(trace_kwargs)
    try:
        res = run_bass_kernel_spmd(_get_nc(), in_maps, list(range(N_CORES)), **kw)
    except Exception:
        # One retry: transient device errors usually clear on re-dispatch.
        res = run_bass_kernel_spmd(_get_nc(), in_maps, list(range(N_CORES)), **kw)
    outs = []
    for b in range(B):
        full = np.concatenate(
            [res.results[b][f"out{s}"] for s in (0, 1, 2)]
            + [res.results[b]["outv"]],
            axis=1,
        )
        outs.append(full.reshape(L, D))
    out = np.stack(outs, axis=0)
    return out.astype(np.float32, copy=False), res


def kernel(**inputs) -> np.ndarray:
    out, _ = _run(inputs, trace=False)
    return out


# revision 25
# speedup vs baseline: 1.7476x; 1.0086x over previous
"""DiffuseRouter kernel for 8 TRN2 NeuronCores.

Reference computation (enable_time=False, soft_time_routing=True):
    out[b, l, d] = (1/3) * sum_g sum_e expert_emb_g[e, b, l, d]
i.e. a uniform-weighted sum of 28 expert planes per batch element.

Sharding: pure data-parallel over batch B=8 -> one batch element per core.
No collectives.

v18 = fp16 load path + PE/DVE column split balanced to their measured
rates.  The harness gate is rel_err < 2e-2; converting the expert planes
to fp16 during host packing HALVES the HBM traffic of this memory-bound
reduction (36.7 -> 18.3 MB per core) at ~7e-4 measured relative error.
PSUM accumulation stays f32; the output is f32.

Measured rates that set the split (v17 trace):
  * DMA: ~26 GB/s per SDMA engine -> 44 us (53 us when engine 15 is in
    its intermittent slow mode) for the 1.15 MB/engine fp16 stream.
  * PE fp16 matmul [128, 512]: ~756 ns (0.68 cols/ns incl. LDWEIGHTS).
  * DVE scalar_tensor_tensor accumulate: 2R1W port-bound at ~0.73
    cols/ns regardless of dtype (single-source ops hit 2.3x that).
  Neither engine alone covers 28 planes x 2560 cols (~105 us); split
  PE=1280 cols (banks 512+512+256) and DVE=1280 cols -> ~54 us each,
  overlapping the 44-53 us load stream.
Carried-over trace laws: only [128, N] contiguous-source dma_starts run
at line rate (partial-partition = half rate, strided = 12 GB/s, SWDGE
hangs); descriptors deal in ceil(n/16) chunks from engine 0; tensor
activity ~44% trips the throttle; per-tile consumption gating means the
final stream tapers to 1-chunk tiles.  Window-major streams let each
PSUM bank stop, scale x1/3 on ACT, and store while later columns load.
"""

import numpy as np

import concourse.bacc as bacc
import concourse.tile as tile
from concourse import mybir
from concourse.alu_op_type import AluOpType
from concourse.bass_utils import run_bass_kernel_spmd

N_CORES = 8
E_TOTAL = 28  # 4 + 8 + 16 experts across the 3 granularity levels
L, D = 256, 1280
P = 128  # SBUF partitions
FD = (L // P) * D  # 2560 free-dim elements per partition per plane
SCALE = 1.0 / 3.0

# Column streams: PE PSUM banks 0/1/2 take cols 0:512/512:1024/1024:1280;
# DVE accumulates cols 1280:2560 in f16.
STREAMS = {0: (0, 512), 1: (512, 1024), 2: (1024, 1280), "V": (1280, 2560)}

# Tile schedule: (stream, chunk_lo, chunk_hi); chunk c of stream s is
# plane c's column block.  Small lead-in tiles start both engines early;
# the final PE stream tapers to 1-chunk tiles (consumption is gated on a
# tile's last descriptor).  All loads are [128, N] contiguous blocks.
# V tiles are front-loaded: the DVE accumulate chain is serial at ~1.46
# us/chunk, so its 28 chunks must all land by ~65% of the stream or the
# chain starves (v18 trace: 14 us of starvation with 1:1 interleave).
SCHED = [
    (0, 0, 4), ("V", 0, 3), ("V", 3, 8), (1, 0, 4), (2, 0, 6),
    ("V", 8, 12), (0, 4, 10), ("V", 12, 16), (1, 4, 12), (2, 6, 14),
    ("V", 16, 20), (0, 10, 18), ("V", 20, 24), (1, 12, 20), (2, 14, 20),
    ("V", 24, 28), (0, 18, 28), (1, 20, 28), (2, 20, 26), (2, 26, 27),
    (2, 27, 28),
]

_NC_CACHE = None


def _build_nc():
    """Build the SPMD Bass program (identical on all 8 cores)."""
    nc = bacc.Bacc(
        "TRN2", target_bir_lowering=False, debug=False, enable_partition_id=False
    )
    f32 = mybir.dt.float32
    f16 = mybir.dt.float16

    xs = []
    for i, (s, lo, hi) in enumerate(SCHED):
        c0, c1 = STREAMS[s]
        xs.append(
            nc.dram_tensor(f"x{i}", [P, (hi - lo) * (c1 - c0)], f16,
                           kind="ExternalInput")
        )
    ident_d = nc.dram_tensor("ident", [P, P], f16, kind="ExternalInput")
    outs_d = {
        s: nc.dram_tensor(f"out{s}", [P, STREAMS[s][1] - STREAMS[s][0]],
                          f32, kind="ExternalOutput")
        for s in STREAMS
        if s != "V"
    }
    outv_d = nc.dram_tensor("outv", [P, STREAMS["V"][1] - STREAMS["V"][0]],
                            f32, kind="ExternalOutput")

    with tile.TileContext(nc) as tc:
        with (
            tc.tile_pool(name="in", bufs=8) as pin,
            tc.tile_pool(name="const", bufs=1) as pconst,
            tc.tile_pool(name="acc", bufs=1) as pacc,
            tc.tile_pool(name="ps", bufs=1, space="PSUM") as pps,
        ):
            ident = pconst.tile([P, P], f16, name="ident", tag="ident")
            # Identity rides the ACT ring; the sync ring carries only loads.
            nc.scalar.dma_start(out=ident[:], in_=ident_d.ap())

            psums = {
                s: pps.tile([P, STREAMS[s][1] - STREAMS[s][0]], f32,
                            name=f"ps{s}", tag=f"ps{s}")
                for s in (0, 1, 2)
            }
            souts = {
                s: pacc.tile([P, STREAMS[s][1] - STREAMS[s][0]], f32,
                             name=f"so{s}", tag=f"so{s}")
                for s in (0, 1, 2)
            }
            vw = STREAMS["V"][1] - STREAMS["V"][0]
            acc = pacc.tile([P, vw], f16, name="acc", tag="acc")
            vout = pacc.tile([P, vw], f32, name="vout", tag="vout")

            mult = AluOpType.mult
            add = AluOpType.add

            with nc.allow_low_precision(
                reason="fp16 DVE accumulation; harness gate is 2e-2"
            ):
                for i, (s, lo, hi) in enumerate(SCHED):
                    w = STREAMS[s][1] - STREAMS[s][0]
                    t = pin.tile([P, (hi - lo) * w], f16)
                    nc.sync.dma_start(out=t[:], in_=xs[i].ap())
                    if s == "V":
                        for c in range(lo, hi):
                            src = t[:, (c - lo) * w : (c - lo + 1) * w]
                            if c == 0:
                                nc.vector.tensor_scalar_mul(acc[:], src, 1.0)
                            else:
                                nc.vector.scalar_tensor_tensor(
                                    acc[:], src, 1.0, acc[:], mult, add
                                )
                        if hi == E_TOTAL:
                            # DVE zone complete: f16 acc -> f32, x1/3, store.
                            nc.scalar.mul(vout[:], acc[:], SCALE)
                            nc.scalar.dma_start(out=outv_d.ap(), in_=vout[:])
                    else:
                        for c in range(lo, hi):
                            nc.tensor.matmul(
                                psums[s][:], ident[:],
                                t[:, (c - lo) * w : (c - lo + 1) * w],
                                start=(c == 0), stop=(c == E_TOTAL - 1),
                            )
                        if hi == E_TOTAL:
                            nc.scalar.mul(souts[s][:], psums[s][:], SCALE)
                            nc.scalar.dma_start(
                                out=outs_d[s].ap(), in_=souts[s][:]
                            )
    nc.compile()
    return nc


def _get_nc():
    global _NC_CACHE
    if _NC_CACHE is None:
        _NC_CACHE = _build_nc()
    return _NC_CACHE


def _pack_core(v16):
    """v16: [28, 128, 2560] fp16 planes for one batch element -> input map."""
    im = {"ident": np.eye(P, dtype=np.float16)}
    for i, (s, lo, hi) in enumerate(SCHED):
        c0, c1 = STREAMS[s]
        blk = v16[lo:hi, :, c0:c1]  # [n, 128, w]
        im[f"x{i}"] = np.ascontiguousarray(
            blk.transpose(1, 0, 2).reshape(P, -1)
        )
    return im


def _run(inputs, trace=False, trace_kwargs=None):
    e0 = np.asarray(inputs["expert_emb_0"], dtype=np.float32)
    e1 = np.asarray(inputs["expert_emb_1"], dtype=np.float32)
    e2 = np.asarray(inputs["expert_emb_2"], dtype=np.float32)
    B = e0.shape[1]
    assert B == N_CORES, f"expected B == {N_CORES}, got {B}"

    in_maps = []
    for b in range(B):
        xb_full = np.concatenate([e0[:, b], e1[:, b], e2[:, b]], axis=0)
        v16 = xb_full.reshape(E_TOTAL, P, FD).astype(np.float16)
        in_maps.append(_pack_core(v16))

    kw = {}
    if trace:
        kw["trace"] = True
        if trace_kwargs:
            kw.update(trace_kwargs)
    try:
        res = run_bass_kernel_spmd(_get_nc(), in_maps, list(range(N_CORES)), **kw)
    except Exception:
        # One retry: transient device errors usually clear on re-dispatch.
        res = run_bass_kernel_spmd(_get_nc(), in_maps, list(range(N_CORES)), **kw)
    outs = []
    for b in range(B):
        full = np.concatenate(
            [res.results[b][f"out{s}"] for s in (0, 1, 2)]
            + [res.results[b]["outv"]],
            axis=1,
        )
        outs.append(full.reshape(L, D))
    out = np.stack(outs, axis=0)
    return out.astype(np.float32, copy=False), res


def kernel(**inputs) -> np.ndarray:
    out, _ = _run(inputs, trace=False)
    return out


# revision 28
# speedup vs baseline: 1.8910x; 1.0820x over previous
"""DiffuseRouter kernel for 8 TRN2 NeuronCores.

Reference computation (enable_time=False, soft_time_routing=True):
    out[b, l, d] = (1/3) * sum_g sum_e expert_emb_g[e, b, l, d]
i.e. a uniform-weighted sum of 28 expert planes per batch element.

Sharding: pure data-parallel over batch B=8 -> one batch element per core.
No collectives.

v18 = fp16 load path + PE/DVE column split balanced to their measured
rates.  The harness gate is rel_err < 2e-2; converting the expert planes
to fp16 during host packing HALVES the HBM traffic of this memory-bound
reduction (36.7 -> 18.3 MB per core) at ~7e-4 measured relative error.
PSUM accumulation stays f32; the output is f32.

Measured rates that set the split (v17 trace):
  * DMA: ~26 GB/s per SDMA engine -> 44 us (53 us when engine 15 is in
    its intermittent slow mode) for the 1.15 MB/engine fp16 stream.
  * PE fp16 matmul [128, 512]: ~756 ns (0.68 cols/ns incl. LDWEIGHTS).
  * DVE scalar_tensor_tensor accumulate: 2R1W port-bound at ~0.73
    cols/ns regardless of dtype (single-source ops hit 2.3x that).
  Neither engine alone covers 28 planes x 2560 cols (~105 us); split
  PE=1280 cols (banks 512+512+256) and DVE=1280 cols -> ~54 us each,
  overlapping the 44-53 us load stream.
Carried-over trace laws: only [128, N] contiguous-source dma_starts run
at line rate (partial-partition = half rate, strided = 12 GB/s, SWDGE
hangs); descriptors deal in ceil(n/16) chunks from engine 0; tensor
activity ~44% trips the throttle; per-tile consumption gating means the
final stream tapers to 1-chunk tiles.  Window-major streams let each
PSUM bank stop, scale x1/3 on ACT, and store while later columns load.
"""

import numpy as np

import concourse.bacc as bacc
import concourse.tile as tile
from concourse import mybir
from concourse.alu_op_type import AluOpType
from concourse.bass_utils import run_bass_kernel_spmd

N_CORES = 8
E_TOTAL = 28  # 4 + 8 + 16 experts across the 3 granularity levels
L, D = 256, 1280
P = 128  # SBUF partitions
FD = (L // P) * D  # 2560 free-dim elements per partition per plane
SCALE = 1.0 / 3.0

# Column streams: PE PSUM banks 0/1/2 take cols 0:512/512:1024/1024:1280;
# DVE accumulates cols 1280:2560 in f16.
STREAMS = {0: (0, 512), 1: (512, 1024), 2: (1024, 1280), "V": (1280, 2560)}

# Tile schedule: (stream, chunk_lo, chunk_hi); chunk c of stream s is
# plane c's column block.  Small lead-in tiles start both engines early;
# the final PE stream tapers to 1-chunk tiles (consumption is gated on a
# tile's last descriptor).  All loads are [128, N] contiguous blocks.
# V tiles are front-loaded: the DVE accumulate chain is serial at ~1.46
# us/chunk, so its 28 chunks must all land by ~65% of the stream or the
# chain starves (v18 trace: 14 us of starvation with 1:1 interleave).
SCHED = [
    (0, 0, 4), ("V", 0, 3), ("V", 3, 8), (1, 0, 4), (2, 0, 6),
    ("V", 8, 12), (0, 4, 10), ("V", 12, 16), (1, 4, 12), (2, 6, 14),
    ("V", 16, 20), (0, 10, 18), ("V", 20, 24), (1, 12, 20), (2, 14, 20),
    ("V", 24, 28), (0, 18, 28), (1, 20, 28), (2, 20, 26), (2, 26, 27),
    (2, 27, 28),
]

_NC_CACHE = None


def _build_nc():
    """Build the SPMD Bass program (identical on all 8 cores)."""
    nc = bacc.Bacc(
        "TRN2", target_bir_lowering=False, debug=False, enable_partition_id=False
    )
    f32 = mybir.dt.float32
    f16 = mybir.dt.float16

    xs = []
    for i, (s, lo, hi) in enumerate(SCHED):
        c0, c1 = STREAMS[s]
        xs.append(
            nc.dram_tensor(f"x{i}", [P, (hi - lo) * (c1 - c0)], f16,
                           kind="ExternalInput")
        )
    ident_d = nc.dram_tensor("ident", [P, P], f16, kind="ExternalInput")
    outs_d = {
        s: nc.dram_tensor(f"out{s}", [P, STREAMS[s][1] - STREAMS[s][0]],
                          f32, kind="ExternalOutput")
        for s in STREAMS
        if s != "V"
    }
    outv_d = nc.dram_tensor("outv", [P, STREAMS["V"][1] - STREAMS["V"][0]],
                            f32, kind="ExternalOutput")

    with tile.TileContext(nc) as tc:
        with (
            tc.tile_pool(name="in", bufs=8) as pin,
            tc.tile_pool(name="const", bufs=1) as pconst,
            tc.tile_pool(name="acc", bufs=1) as pacc,
            tc.tile_pool(name="ps", bufs=1, space="PSUM") as pps,
        ):
            ident = pconst.tile([P, P], f16, name="ident", tag="ident")
            # Identity rides the ACT ring; the sync ring carries only loads.
            nc.scalar.dma_start(out=ident[:], in_=ident_d.ap())

            psums = {
                s: pps.tile([P, STREAMS[s][1] - STREAMS[s][0]], f32,
                            name=f"ps{s}", tag=f"ps{s}")
                for s in (0, 1, 2)
            }
            souts = {
                s: pacc.tile([P, STREAMS[s][1] - STREAMS[s][0]], f32,
                             name=f"so{s}", tag=f"so{s}")
                for s in (0, 1, 2)
            }
            # acc stays f16 (mixed f16-src/f32-acc STT mis-executes), but
            # the x1/3 scale is folded INTO the accumulation so partial
            # sums stay within ~5 where the f16 ulp is 0.004: absmax error
            # drops ~3x vs scaling at the end (1.96e-2 was uncomfortably
            # close to the 2e-2 gate).
            vw = STREAMS["V"][1] - STREAMS["V"][0]
            acc = pacc.tile([P, vw], f16, name="acc", tag="acc")
            vout = pacc.tile([P, vw], f32, name="vout", tag="vout")

            mult = AluOpType.mult
            add = AluOpType.add

            with nc.allow_low_precision(
                reason="fp16 DVE accumulation; harness gate is 2e-2"
            ):
                for i, (s, lo, hi) in enumerate(SCHED):
                    w = STREAMS[s][1] - STREAMS[s][0]
                    t = pin.tile([P, (hi - lo) * w], f16)
                    nc.sync.dma_start(out=t[:], in_=xs[i].ap())
                    if s == "V":
                        for c in range(lo, hi):
                            src = t[:, (c - lo) * w : (c - lo + 1) * w]
                            if c == 0:
                                nc.vector.tensor_scalar_mul(
                                    acc[:], src, SCALE
                                )
                            else:
                                nc.vector.scalar_tensor_tensor(
                                    acc[:], src, SCALE, acc[:], mult, add
                                )
                        if hi == E_TOTAL:
                            # DVE zone complete: f16 acc -> f32 and store
                            # (scale already folded into the chain).
                            nc.scalar.mul(vout[:], acc[:], 1.0)
                            nc.scalar.dma_start(out=outv_d.ap(), in_=vout[:])
                    else:
                        for c in range(lo, hi):
                            nc.tensor.matmul(
                                psums[s][:], ident[:],
                                t[:, (c - lo) * w : (c - lo + 1) * w],
                                start=(c == 0), stop=(c == E_TOTAL - 1),
                            )
                        if hi == E_TOTAL:
                            nc.scalar.mul(souts[s][:], psums[s][:], SCALE)
                            nc.scalar.dma_start(
                                out=outs_d[s].ap(), in_=souts[s][:]
                            )
    nc.compile()
    return nc


def _get_nc():
    global _NC_CACHE
    if _NC_CACHE is None:
        _NC_CACHE = _build_nc()
    return _NC_CACHE


def _pack_core(v16):
    """v16: [28, 128, 2560] fp16 planes for one batch element -> input map."""
    im = {"ident": np.eye(P, dtype=np.float16)}
    for i, (s, lo, hi) in enumerate(SCHED):
        c0, c1 = STREAMS[s]
        blk = v16[lo:hi, :, c0:c1]  # [n, 128, w]
        im[f"x{i}"] = np.ascontiguousarray(
            blk.transpose(1, 0, 2).reshape(P, -1)
        )
    return im


def _run(inputs, trace=False, trace_kwargs=None):
    e0 = np.asarray(inputs["expert_emb_0"], dtype=np.float32)
    e1 = np.asarray(inputs["expert_emb_1"], dtype=np.float32)
    e2 = np.asarray(inputs["expert_emb_2"], dtype=np.float32)
    B = e0.shape[1]
    assert B == N_CORES, f"expected B == {N_CORES}, got {B}"

    in_maps = []
    for b in range(B):
        xb_full = np.concatenate([e0[:, b], e1[:, b], e2[:, b]], axis=0)
        v16 = xb_full.reshape(E_TOTAL, P, FD).astype(np.float16)
        in_maps.append(_pack_core(v16))

    kw = {}
    if trace:
        kw["trace"] = True
        if trace_kwargs:
            kw.update(trace_kwargs)
    try:
        res = run_bass_kernel_spmd(_get_nc(), in_maps, list(range(N_CORES)), **kw)
    except Exception:
        # One retry: transient device errors usually clear on re-dispatch.
        res = run_bass_kernel_spmd(_get_nc(), in_maps, list(range(N_CORES)), **kw)
    outs = []
    for b in range(B):
        full = np.concatenate(
            [res.results[b][f"out{s}"] for s in (0, 1, 2)]
            + [res.results[b]["outv"]],
            axis=1,
        )
        outs.append(full.reshape(L, D))
    out = np.stack(outs, axis=0)
    return out.astype(np.float32, copy=False), res


def kernel(**inputs) -> np.ndarray:
    out, _ = _run(inputs, trace=False)
    return out
